# revision 1
# baseline (speedup 1.0000x reference)
"""Conv2d(256->256, 3x3, pad=1) on 8 TRN2 NeuronCores.

Sharding: data-parallel over output rows (H). Each core computes all 256
output channels for a 28-row slice of the output; the kernel (weights) are
replicated. This keeps the PE array fully loaded (M=128 output channels per
matmul) vs. out-channel sharding which would leave M=32.

Per core the conv is an implicit GEMM: out[o, h, w] = sum over (c, kh, kw) of
xpad[c, h+kh, w+kw] * k[o, c, kh, kw]. Contraction = 2 c-blocks x 9 taps = 18
accumulating matmuls per PSUM tile of [128 o, 2 h-rows x 224 w = 448].
Matmuls run in float32r (fp32 data streamed at bf16 rate, ~1.5e-4 L2 rel err
measured on HW vs fp64 for this contraction depth).
"""

import sys

sys.path.insert(0, "/opt/trn_rl_repo")

import numpy as np

import concourse.mybir as mybir
from concourse import bacc
from concourse.tile import TileContext
from concourse.bass_utils import run_bass_kernel_spmd

N_CORES = 8
C, H, W = 256, 224, 224
O = 256
KH = KW = 3
HS = H // N_CORES          # 28 output rows per core
HROWS = 2                  # output rows per PSUM tile (N = 2*224 = 448)
CB = C // 128              # c blocks
OB = O // 128              # o blocks

_CACHE = {}
LAST_RESULTS = None        # test.py reads exec_time_ns / trace path from here
TRACE = False


def _build():
    nc = bacc.Bacc(None, target_bir_lowering=False)

    xs = nc.dram_tensor(
        "xs", [CB, 128, HS + 2, W + 2], mybir.dt.float32r, kind="ExternalInput"
    )
    w = nc.dram_tensor(
        "w", [CB, 128, KH * KW, O], mybir.dt.float32r, kind="ExternalInput"
    )
    out = nc.dram_tensor(
        "out", [OB, 128, HS, W], mybir.dt.float32, kind="ExternalOutput"
    )

    with TileContext(nc) as tc:
        with (
            tc.tile_pool(name="xin", bufs=1) as px,
            tc.tile_pool(name="win", bufs=1) as pw,
            tc.tile_pool(name="psum", bufs=8, space="PSUM") as pp,
            tc.tile_pool(name="outp", bufs=4) as po,
        ):
            x_sb = []
            w_sb = []
            for b in range(CB):
                xt = px.tile([128, HS + 2, W + 2], mybir.dt.float32r, tag=f"x{b}")
                nc.sync.dma_start(out=xt[:], in_=xs[b])
                wt = pw.tile([128, KH * KW, O], mybir.dt.float32r, tag=f"w{b}")
                nc.sync.dma_start(out=wt[:], in_=w[b])
                x_sb.append(xt)
                w_sb.append(wt)

            n_acc = CB * KH * KW
            for ob in range(OB):
                for h0 in range(0, HS, HROWS):
                    ps = pp.tile([128, HROWS, W], mybir.dt.float32)
                    idx = 0
                    for b in range(CB):
                        for k in range(KH * KW):
                            kh, kw = divmod(k, KW)
                            nc.tensor.matmul(
                                ps[:],
                                w_sb[b][:, k, ob * 128 : (ob + 1) * 128],
                                x_sb[b][:, h0 + kh : h0 + kh + HROWS, kw : kw + W],
                                start=(idx == 0),
                                stop=(idx == n_acc - 1),
                            )
                            idx += 1
                    ot = po.tile([128, HROWS, W], mybir.dt.float32, tag="ot")
                    nc.vector.tensor_copy(out=ot[:], in_=ps[:])
                    nc.sync.dma_start(out=out[ob, :, h0 : h0 + HROWS, :], in_=ot[:])

    nc.compile()
    return nc


def kernel(x: np.ndarray, kernel: np.ndarray) -> np.ndarray:
    global LAST_RESULTS
    if "nc" not in _CACHE:
        _CACHE["nc"] = _build()
    nc = _CACHE["nc"]

    x = np.ascontiguousarray(x, dtype=np.float32)
    kw_arr = np.ascontiguousarray(kernel, dtype=np.float32)

    xp = np.pad(x, ((0, 0), (1, 1), (1, 1)))          # [C, H+2, W+2]
    xp = xp.reshape(CB, 128, H + 2, W + 2)
    # w_t[b, p, kh*KW+kw, o] = kernel[o, b*128+p, kh, kw]
    w_t = np.ascontiguousarray(
        kw_arr.transpose(1, 2, 3, 0).reshape(CB, 128, KH * KW, O)
    )

    in_maps = []
    for i in range(N_CORES):
        xs_i = np.ascontiguousarray(xp[:, :, i * HS : i * HS + HS + 2, :])
        in_maps.append({"xs": xs_i, "w": w_t})

    results = run_bass_kernel_spmd(
        nc, in_maps, core_ids=list(range(N_CORES)), trace=TRACE
    )
    LAST_RESULTS = results

    parts = [r["out"].reshape(O, HS, W) for r in results.results]
    return np.concatenate(parts, axis=1)


# revision 4
# speedup vs baseline: 1.0329x; 1.0329x over previous
"""Conv2d(256->256, 3x3, pad=1) on 8 TRN2 NeuronCores.

Sharding: data-parallel over output rows (H). Each core computes all 256
output channels for a 28-row slice of the output; the kernel (weights) are
replicated. This keeps the PE array fully loaded (M=128 output channels per
matmul) vs. out-channel sharding which would leave M=32.

Per core the conv is an implicit GEMM: out[o, h, w] = sum over (c, kh, kw) of
xpad[c, h+kh, w+kw] * k[o, c, kh, kw]. Contraction = 2 c-blocks x 9 taps = 18
accumulating matmuls per PSUM tile of [128 o, 2 h-rows x 224 w = 448].
Matmuls run in float32r (fp32 data streamed at bf16 rate, ~1.5e-4 L2 rel err
measured on HW vs fp64 for this contraction depth).
"""

import sys

sys.path.insert(0, "/opt/trn_rl_repo")

import numpy as np

import concourse.mybir as mybir
from concourse import bacc
from concourse.tile import TileContext
from concourse.bass_utils import run_bass_kernel_spmd

N_CORES = 8
C, H, W = 256, 224, 224
O = 256
KH = KW = 3
HS = H // N_CORES          # 28 output rows per core
HROWS = 2                  # output rows per PSUM tile (N = 2*224 = 448)
CB = C // 128              # c blocks
OB = O // 128              # o blocks

_CACHE = {}
LAST_RESULTS = None        # test.py reads exec_time_ns / trace path from here
TRACE = False


def _build():
    nc = bacc.Bacc(None, target_bir_lowering=False)

    xs = nc.dram_tensor(
        "xs", [CB, 128, HS + 2, W + 2], mybir.dt.float32r, kind="ExternalInput"
    )
    w = nc.dram_tensor(
        "w", [CB, 128, KH * KW, O], mybir.dt.float32r, kind="ExternalInput"
    )
    out = nc.dram_tensor(
        "out", [OB, 128, HS, W], mybir.dt.float32, kind="ExternalOutput"
    )

    with TileContext(nc) as tc:
        with (
            tc.tile_pool(name="warm", bufs=1) as pwarm,
            tc.tile_pool(name="win", bufs=1) as pw,
            tc.tile_pool(name="xwin", bufs=6) as px,
            tc.tile_pool(name="psumw", bufs=1, space="PSUM") as ppw,
            tc.tile_pool(name="psum", bufs=7, space="PSUM") as pp,
            tc.tile_pool(name="outp", bufs=4) as po,
        ):
            # PE warmup: dummy matmuls on a memset tile while input DMAs
            # stream, so the HAM clock-gate is at 8/8 when real work starts.
            wt0 = pwarm.tile([128, 128], mybir.dt.bfloat16, tag="warm")
            ps0 = ppw.tile([128, 128], mybir.dt.float32, tag="warmps")
            nc.vector.memset(wt0[:], 0.0)
            for _ in range(32):
                nc.tensor.matmul(ps0[:], wt0[:], wt0[:], start=True, stop=True)

            w_sb = []
            for b in range(CB):
                wt = pw.tile([128, KH * KW, O], mybir.dt.float32r, tag=f"w{b}")
                nc.sync.dma_start(out=wt[:], in_=w[b])
                w_sb.append(wt)

            n_acc = CB * KH * KW
            for h0 in range(0, HS, HROWS):
                xw = []
                for b in range(CB):
                    xt = px.tile(
                        [128, HROWS + 2, W + 2], mybir.dt.float32r, tag=f"xw{b}"
                    )
                    nc.sync.dma_start(out=xt[:], in_=xs[b, :, h0 : h0 + HROWS + 2, :])
                    xw.append(xt)
                for ob in range(OB):
                    ps = pp.tile([128, HROWS, W], mybir.dt.float32, tag="ps")
                    idx = 0
                    for b in range(CB):
                        for k in range(KH * KW):
                            kh, kw = divmod(k, KW)
                            nc.tensor.matmul(
                                ps[:],
                                w_sb[b][:, k, ob * 128 : (ob + 1) * 128],
                                xw[b][:, kh : kh + HROWS, kw : kw + W],
                                start=(idx == 0),
                                stop=(idx == n_acc - 1),
                            )
                            idx += 1
                    ot = po.tile([128, HROWS, W], mybir.dt.float32, tag="ot")
                    nc.vector.tensor_copy(out=ot[:], in_=ps[:])
                    nc.sync.dma_start(out=out[ob, :, h0 : h0 + HROWS, :], in_=ot[:])

    nc.compile()
    return nc


def kernel(x: np.ndarray, kernel: np.ndarray) -> np.ndarray:
    global LAST_RESULTS
    if "nc" not in _CACHE:
        _CACHE["nc"] = _build()
    nc = _CACHE["nc"]

    x = np.ascontiguousarray(x, dtype=np.float32)
    kw_arr = np.ascontiguousarray(kernel, dtype=np.float32)

    xp = np.pad(x, ((0, 0), (1, 1), (1, 1)))          # [C, H+2, W+2]
    xp = xp.reshape(CB, 128, H + 2, W + 2)
    # w_t[b, p, kh*KW+kw, o] = kernel[o, b*128+p, kh, kw]
    w_t = np.ascontiguousarray(
        kw_arr.transpose(1, 2, 3, 0).reshape(CB, 128, KH * KW, O)
    )

    in_maps = []
    for i in range(N_CORES):
        xs_i = np.ascontiguousarray(xp[:, :, i * HS : i * HS + HS + 2, :])
        in_maps.append({"xs": xs_i, "w": w_t})

    results = run_bass_kernel_spmd(
        nc, in_maps, core_ids=list(range(N_CORES)), trace=TRACE
    )
    LAST_RESULTS = results

    parts = [r["out"].reshape(O, HS, W) for r in results.results]
    return np.concatenate(parts, axis=1)


# revision 7
# speedup vs baseline: 1.0661x; 1.0322x over previous
"""Conv2d(256->256, 3x3, pad=1) on 8 TRN2 NeuronCores.

Sharding: data-parallel over output rows (H). Each core computes all 256
output channels for a 28-row slice of the output; the kernel (weights) are
replicated. This keeps the PE array fully loaded (M=128 output channels per
matmul) vs. out-channel sharding which would leave M=32.

Per core the conv is an implicit GEMM: out[o, h, w] = sum over (c, kh, kw) of
xpad[c, h+kh, w+kw] * k[o, c, kh, kw]. Contraction = 2 c-blocks x 9 taps = 18
accumulating matmuls per PSUM tile of [128 o, 2 h-rows x 224 w = 448].
Matmuls run in float32r (fp32 data streamed at bf16 rate, ~1.5e-4 L2 rel err
measured on HW vs fp64 for this contraction depth).
"""

import sys

sys.path.insert(0, "/opt/trn_rl_repo")

import numpy as np

import concourse.mybir as mybir
from concourse import bacc
from concourse.tile import TileContext
from concourse.bass_utils import run_bass_kernel_spmd

N_CORES = 8
C, H, W = 256, 224, 224
O = 256
KH = KW = 3
HS = H // N_CORES          # 28 output rows per core
HROWS = 2                  # output rows per PSUM tile (N = 2*224 = 448)
CB = C // 128              # c blocks
OB = O // 128              # o blocks

_CACHE = {}
LAST_RESULTS = None        # test.py reads exec_time_ns / trace path from here
TRACE = False


def _build():
    nc = bacc.Bacc(None, target_bir_lowering=False)

    xs = nc.dram_tensor(
        "xs", [CB, 128, HS + 2, W + 2], mybir.dt.float32r, kind="ExternalInput"
    )
    w = nc.dram_tensor(
        "w", [CB, 128, KH * KW, O], mybir.dt.float32r, kind="ExternalInput"
    )
    out = nc.dram_tensor(
        "out", [OB, 128, HS, W], mybir.dt.float32, kind="ExternalOutput"
    )

    n_warm = 36
    with TileContext(nc) as tc:
        with (
            tc.tile_pool(name="warm", bufs=1) as pwarm,
            tc.tile_pool(name="win", bufs=1) as pw,
            tc.tile_pool(name="xwin", bufs=8) as px,
            tc.tile_pool(name="psumw", bufs=1, space="PSUM") as ppw,
            tc.tile_pool(name="psum", bufs=7, space="PSUM") as pp,
            tc.tile_pool(name="outp", bufs=4) as po,
        ):
            # PE warmup: dummy matmuls on a memset tile while input DMAs
            # stream, so the HAM clock-gate is at 8/8 when real work starts.
            wt0 = pwarm.tile([128, 256], mybir.dt.bfloat16, tag="warm")
            ps0 = ppw.tile([128, 256], mybir.dt.float32, tag="warmps")
            nc.vector.memset(wt0[:], 0.0)
            for _ in range(n_warm):
                nc.tensor.matmul(ps0[:], wt0[:, :128], wt0[:], start=True, stop=True)

            # Weights in (b, ob) quarters; the first PSUM group consumes
            # (b=0,ob=0) then (b=1,ob=0), so order the gate DMAs that way.
            w_sb = [
                pw.tile(
                    [128, KH * KW, O], mybir.dt.float32r, tag=f"w{b}", name=f"w{b}"
                )
                for b in range(CB)
            ]
            xw0 = []
            for b in range(CB):
                nc.sync.dma_start(
                    out=w_sb[b][:, :, 0:128], in_=w[b, :, :, 0:128]
                )
                xt = px.tile([128, HROWS + 2, W + 2], mybir.dt.float32r, tag=f"xw{b}")
                nc.sync.dma_start(out=xt[:], in_=xs[b, :, 0 : HROWS + 2, :])
                xw0.append(xt)
            for b in range(CB):
                nc.sync.dma_start(
                    out=w_sb[b][:, :, 128:256], in_=w[b, :, :, 128:256]
                )

            n_acc = CB * KH * KW
            for h0 in range(0, HS, HROWS):
                if h0 == 0:
                    xw = xw0
                else:
                    xw = []
                    for b in range(CB):
                        xt = px.tile(
                            [128, HROWS + 2, W + 2], mybir.dt.float32r, tag=f"xw{b}"
                        )
                        nc.sync.dma_start(
                            out=xt[:], in_=xs[b, :, h0 : h0 + HROWS + 2, :]
                        )
                        xw.append(xt)
                for ob in range(OB):
                    ps = pp.tile([128, HROWS, W], mybir.dt.float32, tag="ps")
                    idx = 0
                    for b in range(CB):
                        for k in range(KH * KW):
                            kh, kw = divmod(k, KW)
                            nc.tensor.matmul(
                                ps[:],
                                w_sb[b][:, k, ob * 128 : (ob + 1) * 128],
                                xw[b][:, kh : kh + HROWS, kw : kw + W],
                                start=(idx == 0),
                                stop=(idx == n_acc - 1),
                            )
                            idx += 1
                    ot = po.tile([128, HROWS, W], mybir.dt.float32, tag="ot")
                    nc.vector.tensor_copy(out=ot[:], in_=ps[:])
                    nc.sync.dma_start(out=out[ob, :, h0 : h0 + HROWS, :], in_=ot[:])

    nc.compile()
    return nc


def kernel(x: np.ndarray, kernel: np.ndarray) -> np.ndarray:
    global LAST_RESULTS
    if "nc" not in _CACHE:
        _CACHE["nc"] = _build()
    nc = _CACHE["nc"]

    x = np.ascontiguousarray(x, dtype=np.float32)
    kw_arr = np.ascontiguousarray(kernel, dtype=np.float32)

    xp = np.pad(x, ((0, 0), (1, 1), (1, 1)))          # [C, H+2, W+2]
    xp = xp.reshape(CB, 128, H + 2, W + 2)
    # w_t[b, p, kh*KW+kw, o] = kernel[o, b*128+p, kh, kw]
    w_t = np.ascontiguousarray(
        kw_arr.transpose(1, 2, 3, 0).reshape(CB, 128, KH * KW, O)
    )

    in_maps = []
    for i in range(N_CORES):
        xs_i = np.ascontiguousarray(xp[:, :, i * HS : i * HS + HS + 2, :])
        in_maps.append({"xs": xs_i, "w": w_t})

    results = run_bass_kernel_spmd(
        nc, in_maps, core_ids=list(range(N_CORES)), trace=TRACE
    )
    LAST_RESULTS = results

    parts = [r["out"].reshape(O, HS, W) for r in results.results]
    return np.concatenate(parts, axis=1)
